# revision 6
# baseline (speedup 1.0000x reference)
"""Autoregressive LSTM decompressor on 8 Trainium2 NeuronCores, v2.

Math (from the reference): the output h of each step feeds back as the next
step's input, so for t>=1 the two matmuls collapse into one with
W = W_ih + W_hh:
    gates_0 = x @ W_ih.T + b            (h0 = c0 = 0; done on HOST, one matvec)
    gates_t = h_t @ W.T + b             (device, steps t=1..255)
    i,f,g,o = split(gates); c' = sig(f)*c + sig(i)*tanh(g); h' = sig(o)*tanh(c')
    y = stack(h_1..h_256) @ W_out.T + b_out

Device strategy (per step, tensor-parallel over the 4D gate dim):
  * Core r owns hidden units [256r, 256r+256) and the 1024 matching gate rows.
  * Gates are computed PARTITION-major: 136 matmuls with the weight tile
    [128h x 128gate] as the (free) stationary operand and the h chunk [128,1]
    as the moving operand -> psum [128, 8] = cols [i0 i1 f0 f1 o0 o1 g0 g1].
    Bias rides as a 17th contraction chunk against an e0 column.
  * Elementwise LSTM cell on [128,2] tiles across scalar/vector/pool engines.
  * h exchange: 8 single-destination remote_dma_broadcast sends per step
    (slot j -> physical peer my_tpb^j). Receiver-side slot s therefore holds
    the h chunk of logical core (s ^ r); each core's weight layout is
    permuted host-side to match. SBUF->SBUF, ~0.3us vs ~15us for a
    collective_compute AllGather. Per-slot arrival semaphores keep the
    consumer exact. hist slots are written once per step, never reused.
  * Final projection y = H @ W_out.T is sharded over output columns
    (core r computes douts [128r, 128r+128)).

Host-side prep is input formatting plus the single step-0 matvec (0.4% of
the FLOPs); all 255 recurrent steps and the projection run on device.
"""

import numpy as np
import ml_dtypes

D = 2048           # hidden width
DOUT = 1024        # output width
L = 256            # seq_len
NC = 8             # cores
KC = 16            # 128-wide contraction chunks per step
NSTEP = L - 1      # device steps (step 0 on host)

_BF16 = ml_dtypes.bfloat16

# psum column m -> gate type (rows in W are [i | f | g | o] blocks of 2048)
# col order   [i0 i1 f0 f1 o0 o1 g0 g1]  (sigmoid group contiguous 0:6)
_GT = np.array([0, 0, 1, 1, 3, 3, 2, 2])


# slot -> sender XOR map measured on HW: receiver r's hist slot-pair j holds
# the h chunk of logical core (r ^ SX[j]).  (Cross-die sends swap bit 1:
# the runtime's logical->physical NC map is the linear bit-matrix
# [bit2, bit1^bit2, bit0].)
SX = np.array([0, 1, 2, 3, 6, 7, 4, 5])


def _unit_map(r):
    """[128,16] global hidden unit of (partition p, hist chunk q) on core r."""
    P = np.arange(128)
    Q = np.arange(KC)
    gch = 2 * (SX[Q // 2] ^ r) + (Q % 2)        # global 128-chunk id
    return gch[None, :] * 128 + P[:, None]


def _row_map(r):
    """[8,128] W row of (psum col m, psum partition g) on core r."""
    M = np.arange(8)
    G = np.arange(128)
    return (2048 * _GT[M][:, None] + 256 * r + 128 * (M % 2)[:, None]
            + G[None, :])


def _prep_core_inputs(x, W_ih, W_sum, b, W_out):
    """Host: step-0 LSTM cell + per-core permuted weight uploads."""
    # step 0 on host in fp32 (h0 = c0 = 0)
    g0 = W_ih @ x + b
    i0, f0, gg0, o0 = np.split(g0, 4)
    c1 = _sig(i0) * np.tanh(gg0)
    h1 = _sig(o0) * np.tanh(c1)

    in_maps = []
    for r in range(NC):
        um = _unit_map(r)                       # [128,16]
        rm = _row_map(r)                        # [8,128]
        wrec = np.zeros((128, KC + 1, 8, 128), np.float32)
        # wrec[p,q,m,g] = W_sum[rm[m,g], um[p,q]]
        wsel = W_sum[rm.reshape(-1)][:, um.reshape(-1)]   # [1024, 2048]
        wrec[:, :KC] = (wsel.reshape(8, 128, 128, KC)
                        .transpose(2, 3, 0, 1))
        wrec[0, KC] = b[rm]                     # bias chunk (vs e0 column)
        wout = np.transpose(
            W_out[128 * r:128 * r + 128][:, um.reshape(-1)]
            .reshape(128, 128, KC), (1, 2, 0))  # [p, q, d]
        in_maps.append({
            "wrec": wrec.astype(_BF16),
            "wout": np.ascontiguousarray(wout).astype(_BF16),
            "h1": h1[um].astype(_BF16),         # [128,16]
            "c1": np.stack([c1[256 * r + np.arange(128)],
                            c1[256 * r + 128 + np.arange(128)]],
                           axis=1).astype(np.float32),    # [128,2]
            "e0": np.eye(128, 1, dtype=np.float32).astype(_BF16),
        })
    return in_maps


def _sig(v):
    return 1.0 / (1.0 + np.exp(-v))


def _build_program(nsteps=NSTEP):
    from concourse import bacc, mybir, library_config
    from contextlib import ExitStack

    dt = mybir.dt
    Sig = mybir.ActivationFunctionType.Sigmoid
    Tanh = mybir.ActivationFunctionType.Tanh

    nc = bacc.Bacc("TRN2", target_bir_lowering=False, debug=False,
                   num_devices=NC, num_swdge_queues=2)
    nc.detect_race_conditions = False

    wrec_d = nc.dram_tensor("wrec", [128, KC + 1, 8, 128], dt.bfloat16,
                            kind="ExternalInput")
    wout_d = nc.dram_tensor("wout", [128, KC, 128], dt.bfloat16,
                            kind="ExternalInput")
    h1_d = nc.dram_tensor("h1", [128, KC], dt.bfloat16, kind="ExternalInput")
    c1_d = nc.dram_tensor("c1", [128, 2], dt.float32, kind="ExternalInput")
    e0_d = nc.dram_tensor("e0", [128, 1], dt.bfloat16, kind="ExternalInput")
    y_d = nc.dram_tensor("y", [L, 128], dt.float32, kind="ExternalOutput")

    stack = ExitStack()
    ec = stack.enter_context
    wrec = ec(nc.sbuf_tensor("wrec_sb", [128, KC + 1, 8, 128], dt.bfloat16))
    wout = ec(nc.sbuf_tensor("wout_sb", [128, KC, 128], dt.bfloat16))
    hist = ec(nc.sbuf_tensor("hist_sb", [128, L, KC], dt.bfloat16))
    e0 = ec(nc.sbuf_tensor("e0_sb", [128, 1], dt.bfloat16))
    sigall = ec(nc.sbuf_tensor("sigall", [128, 6], dt.float32))
    tgc = ec(nc.sbuf_tensor("tgc", [128, 4], dt.float32))
    m12 = ec(nc.sbuf_tensor("m12", [128, 4], dt.float32))
    tcn = ec(nc.sbuf_tensor("tcn", [128, 2], dt.float32))
    hbuf = ec(nc.sbuf_tensor("hbuf", [128, 2, 2], dt.bfloat16))
    ysb = ec(nc.sbuf_tensor("ysb", [128, 256], dt.float32))
    pp0 = ec(nc.psum_tensor("pp0", [128, 8], dt.float32))
    pp1 = ec(nc.psum_tensor("pp1", [128, 8], dt.float32))
    yp = ec(nc.psum_tensor("yp", [128, 128], dt.float32))
    dsem = ec(nc.semaphore("dsem"))     # initial DMAs (HWDGE, x16)
    mmsem = ec(nc.semaphore("mmsem"))   # psum gates ready,   +1/step
    sa1 = ec(nc.semaphore("sa1"))       # sig+tanh_g ready,   +1/step
    sa2 = ec(nc.semaphore("sa2"))       # tanh_c ready,       +1/step
    sv1 = ec(nc.semaphore("sv1"))       # m12 ready,          +1/step
    sv2 = ec(nc.semaphore("sv2"))       # c updated,          +1/step
    sp1 = ec(nc.semaphore("sp1"))       # h written,          +1/step
    lsem0 = ec(nc.semaphore("lsem0"))   # queue-0 sends drained, +64/step
    lsem1 = ec(nc.semaphore("lsem1"))   # queue-1 sends drained, +64/step
    psem = ec(nc.semaphore("psem"))     # descs generated,    +8/step
    vfin = ec(nc.semaphore("vfin"))     # projection copies
    with nc.Block() as block:
        rsems = [stack.enter_context(nc.semaphore(f"rs{j}"))
                 for j in range(NC)]      # slot-j arrival, +2/step
        pp = [pp0, pp1]

        @block.tensor
        def _(te):
            te.wait_ge(dsem, 96)          # ALL initial DMAs (both wrec halves)
            for t in range(1, nsteps + 1):
                ps = pp[t % 2]
                if t >= 3:
                    te.wait_ge(sa1, t - 2)     # psum bank consumed (t-2)
                for m in range(8):             # bias chunk first (no dep)
                    te.matmul(ps[:, m:m + 1], wrec[:, KC, m, :], e0[:, 0:1],
                              start=(m == 0), stop=False)
                for j in range(NC):
                    if t >= 2:
                        te.wait_ge(rsems[j], 2 * (t - 1))
                    for i in range(2):
                        q = 2 * j + i
                        for m in range(8):
                            mm = te.matmul(
                                ps[:, m:m + 1], wrec[:, q, m, :],
                                hist[:, t - 1, q:q + 1],
                                start=False,
                                stop=(j == 7 and i == 1 and m == 7))
                mm.then_inc(mmsem, 1)
            # final projection: y[:, 128r:128r+128]
            for j in range(NC):
                te.wait_ge(rsems[j], 2 * nsteps)
            for tb in range(2):
                for q in range(KC):
                    mm = te.matmul(yp[:, :],
                                   hist[:, 128 * tb:128 * (tb + 1), q],
                                   wout[:, q, :],
                                   start=(q == 0), stop=(q == KC - 1))
                mm.then_inc(mmsem, 1)
                te.wait_ge(vfin, tb + 1)   # psum copied before reuse

        @block.scalar
        def _(sc):
            sc.dma_start(wrec[:, 9:17], wrec_d[:, 9:17]).then_inc(dsem, 16)
            for t in range(1, nsteps + 1):
                sc.wait_ge(mmsem, t)
                if t >= 2:
                    sc.wait_ge(sv1, t - 1)     # tgc[0:2] consumed by mul
                    sc.wait_ge(sp1, t - 1)     # sigall/tcn consumed by h-mul
                ps = pp[t % 2]
                sc.activation(sigall[:, :], ps[:, 0:6], Sig)
                sc.activation(tgc[:, 0:2], ps[:, 6:8], Tanh).then_inc(sa1, 1)
                sc.wait_ge(sv1, t)             # m12 ready
                sc.activation(tcn[:, 0:1], m12[:, 0:1], Tanh,
                              bias=m12[:, 2:3])
                sc.activation(tcn[:, 1:2], m12[:, 1:2], Tanh,
                              bias=m12[:, 3:4]).then_inc(sa2, 1)

        @block.vector
        def _(ve):
            ve.wait_ge(dsem, 96)               # all initial DMAs done
            for t in range(1, nsteps + 1):
                ve.wait_ge(sa1, t)
                if t >= 2:
                    ve.wait_ge(sv2, t - 1)     # same-engine RAW on tgc[2:4]
                    ve.wait_ge(sa2, t - 1)     # m12 consumed by tanh_c
                ve.tensor_mul(m12[:, :], sigall[:, 0:4], tgc[:, :]) \
                  .then_inc(sv1, 1)            # [i*tg | f*c]
                ve.wait_ge(sv1, t)             # same-engine RAW on m12
                ve.tensor_add(tgc[:, 2:4], m12[:, 0:2], m12[:, 2:4]) \
                  .then_inc(sv2, 1)            # c update (next step's f*c)
            # projection psum -> sbuf
            for tb in range(2):
                ve.wait_ge(mmsem, nsteps + tb + 1)
                ve.tensor_copy(ysb[:, 128 * tb:128 * (tb + 1)], yp[:, :]) \
                  .then_inc(vfin, 1)

        @block.gpsimd
        def _(gp):
            gp.load_library(library_config.remote_dma)
            for t in range(1, nsteps + 1):
                # desc-gen for this step's 8 sends (addresses only; runs
                # during the matvec). Slots 0-3 on queue 0, 4-7 on queue 1.
                for j in range(NC):
                    dests = [None] * NC
                    dests[j] = (0, j)
                    gp.remote_dma_broadcast(
                        hist[:, t, 2 * j:2 * j + 2],
                        hbuf[:, t % 2, :],
                        remote_sem=rsems[j],
                        local_sem=(lsem0 if j < 4 else lsem1),
                        rdests=dests,
                        queue_num=j // 4,
                    ).then_inc(psem, 1)
                # h = sig_o * tanh_c  (on Pool so the triggers that follow
                # need no cross-engine handshake)
                if t >= 3:
                    gp.wait_ge(lsem0, 64 * (t - 2))   # hbuf[t%2] drained
                    gp.wait_ge(lsem1, 64 * (t - 2))
                gp.wait_ge(sa2, t)
                gp.tensor_mul(hbuf[:, t % 2, :], sigall[:, 4:6], tcn[:, :]) \
                  .then_inc(sp1, 1)
                gp.wait_ge(sp1, t)      # all Q7 lanes of the mul committed
                gp.wait_ge(psem, 8 * t)
                gp.trigger_dma(count=4, queue_num=0)
                gp.trigger_dma(count=4, queue_num=1)

        @block.sync
        def _(sy):
            sy.dma_start(wrec[:, 0:9], wrec_d[:, 0:9]).then_inc(dsem, 16)
            sy.dma_start(e0[:], e0_d[:]).then_inc(dsem, 16)
            sy.dma_start(hist[:, 0, :], h1_d[:]).then_inc(dsem, 16)
            sy.dma_start(tgc[:, 2:4], c1_d[:]).then_inc(dsem, 16)
            sy.dma_start(wout[:], wout_d[:]).then_inc(dsem, 16)
            sy.wait_ge(vfin, 2)
            sy.dma_start(y_d[0:128, :], ysb[:, 0:128]).then_inc(dsem, 16)
            sy.dma_start(y_d[128:256, :], ysb[:, 128:256]).then_inc(dsem, 16)
            sy.wait_ge(dsem, 128)

    nc.compile()
    return nc


def kernel(x, W_ih, W_hh, b_ih, b_hh, W_out, b_out, seq_len, _trace=False):
    from concourse.bass_utils import run_bass_kernel_spmd

    assert int(seq_len) == L
    x = np.asarray(x, np.float32)[0]
    W_ih = np.asarray(W_ih, np.float32)
    W_sum = W_ih + np.asarray(W_hh, np.float32)
    b = np.asarray(b_ih, np.float32) + np.asarray(b_hh, np.float32)
    W_out = np.asarray(W_out, np.float32)
    b_out = np.asarray(b_out, np.float32)

    in_maps = _prep_core_inputs(x, W_ih, W_sum, b, W_out)
    nc = _build_program()
    res = run_bass_kernel_spmd(nc, in_maps, list(range(NC)), trace=_trace)
    y = np.concatenate(
        [np.asarray(res.results[r]["y"], np.float32) for r in range(NC)],
        axis=1)
    out = (y + b_out)[None]     # [1, L, DOUT]
    if _trace:
        return out, res
    return out


# revision 7
# speedup vs baseline: 1.6930x; 1.6930x over previous
"""Autoregressive LSTM decompressor on 8 Trainium2 NeuronCores, v2.

Math (from the reference): the output h of each step feeds back as the next
step's input, so for t>=1 the two matmuls collapse into one with
W = W_ih + W_hh:
    gates_0 = x @ W_ih.T + b            (h0 = c0 = 0; done on HOST, one matvec)
    gates_t = h_t @ W.T + b             (device, steps t=1..255)
    i,f,g,o = split(gates); c' = sig(f)*c + sig(i)*tanh(g); h' = sig(o)*tanh(c')
    y = stack(h_1..h_256) @ W_out.T + b_out

Device strategy (per step, tensor-parallel over the 4D gate dim):
  * Core r owns hidden units [256r, 256r+256) and the 1024 matching gate rows.
  * Gates are computed PARTITION-major: 136 matmuls with the weight tile
    [128h x 128gate] as the (free) stationary operand and the h chunk [128,1]
    as the moving operand -> psum [128, 8] = cols [i0 i1 f0 f1 o0 o1 g0 g1].
    Bias rides as a 17th contraction chunk against an e0 column.
  * Elementwise LSTM cell on [128,2] tiles across scalar/vector/pool engines.
  * h exchange: 8 single-destination remote_dma_broadcast sends per step
    (slot j -> physical peer my_tpb^j). Receiver-side slot s therefore holds
    the h chunk of logical core (s ^ r); each core's weight layout is
    permuted host-side to match. SBUF->SBUF, ~0.3us vs ~15us for a
    collective_compute AllGather. Per-slot arrival semaphores keep the
    consumer exact. hist slots are written once per step, never reused.
  * Final projection y = H @ W_out.T is sharded over output columns
    (core r computes douts [128r, 128r+128)).

Host-side prep is input formatting plus the single step-0 matvec (0.4% of
the FLOPs); all 255 recurrent steps and the projection run on device.
"""

import numpy as np
import ml_dtypes

D = 2048           # hidden width
DOUT = 1024        # output width
L = 256            # seq_len
NC = 8             # cores
KC = 16            # 128-wide contraction chunks per step
NSTEP = L - 1      # device steps (step 0 on host)

_BF16 = ml_dtypes.bfloat16

# psum column m -> gate type (rows in W are [i | f | g | o] blocks of 2048)
# col order   [i0 i1 f0 f1 o0 o1 g0 g1]  (sigmoid group contiguous 0:6)
_GT = np.array([0, 0, 1, 1, 3, 3, 2, 2])


# slot -> sender XOR map measured on HW: receiver r's hist slot-pair j holds
# the h chunk of logical core (r ^ SX[j]).  (Cross-die sends swap bit 1:
# the runtime's logical->physical NC map is the linear bit-matrix
# [bit2, bit1^bit2, bit0].)
SX = np.array([0, 1, 2, 3, 6, 7, 4, 5])


def _unit_map(r):
    """[128,16] global hidden unit of (partition p, hist chunk q) on core r."""
    P = np.arange(128)
    Q = np.arange(KC)
    gch = 2 * (SX[Q // 2] ^ r) + (Q % 2)        # global 128-chunk id
    return gch[None, :] * 128 + P[:, None]


def _row_map(r):
    """[8,128] W row of (psum col m, psum partition g) on core r."""
    M = np.arange(8)
    G = np.arange(128)
    return (2048 * _GT[M][:, None] + 256 * r + 128 * (M % 2)[:, None]
            + G[None, :])


def _prep_core_inputs(x, W_ih, W_sum, b, W_out):
    """Host: step-0 LSTM cell + per-core permuted weight uploads."""
    # step 0 on host in fp32 (h0 = c0 = 0)
    g0 = W_ih @ x + b
    i0, f0, gg0, o0 = np.split(g0, 4)
    c1 = _sig(i0) * np.tanh(gg0)
    h1 = _sig(o0) * np.tanh(c1)

    in_maps = []
    for r in range(NC):
        um = _unit_map(r)                       # [128,16]
        rm = _row_map(r)                        # [8,128]
        wrec = np.zeros((128, KC + 1, 8, 128), np.float32)
        # wrec[p,q,m,g] = W_sum[rm[m,g], um[p,q]]
        wsel = W_sum[rm.reshape(-1)][:, um.reshape(-1)]   # [1024, 2048]
        wrec[:, :KC] = (wsel.reshape(8, 128, 128, KC)
                        .transpose(2, 3, 0, 1))
        wrec[0, KC] = b[rm]                     # bias chunk (vs e0 column)
        wout = np.transpose(
            W_out[128 * r:128 * r + 128][:, um.reshape(-1)]
            .reshape(128, 128, KC), (1, 2, 0))  # [p, q, d]
        in_maps.append({
            "wrec": wrec.astype(_BF16),
            "wout": np.ascontiguousarray(wout).astype(_BF16),
            "h1": h1[um].astype(_BF16),         # [128,16]
            "c1": np.stack([c1[256 * r + np.arange(128)],
                            c1[256 * r + 128 + np.arange(128)]],
                           axis=1).astype(np.float32),    # [128,2]
            "e0": np.eye(128, 1, dtype=np.float32).astype(_BF16),
        })
    return in_maps


def _sig(v):
    return 1.0 / (1.0 + np.exp(-v))


def _build_program(nsteps=NSTEP):
    from concourse import bacc, mybir, library_config
    from contextlib import ExitStack

    dt = mybir.dt
    Sig = mybir.ActivationFunctionType.Sigmoid
    Tanh = mybir.ActivationFunctionType.Tanh

    nc = bacc.Bacc("TRN2", target_bir_lowering=False, debug=False,
                   num_devices=NC, num_swdge_queues=2)
    nc.detect_race_conditions = False

    wrec_d = nc.dram_tensor("wrec", [128, KC + 1, 8, 128], dt.bfloat16,
                            kind="ExternalInput")
    wout_d = nc.dram_tensor("wout", [128, KC, 128], dt.bfloat16,
                            kind="ExternalInput")
    h1_d = nc.dram_tensor("h1", [128, KC], dt.bfloat16, kind="ExternalInput")
    c1_d = nc.dram_tensor("c1", [128, 2], dt.float32, kind="ExternalInput")
    e0_d = nc.dram_tensor("e0", [128, 1], dt.bfloat16, kind="ExternalInput")
    y_d = nc.dram_tensor("y", [L, 128], dt.float32, kind="ExternalOutput")

    stack = ExitStack()
    ec = stack.enter_context
    wrec = ec(nc.sbuf_tensor("wrec_sb", [128, KC + 1, 8, 128], dt.bfloat16))
    wout = ec(nc.sbuf_tensor("wout_sb", [128, KC, 128], dt.bfloat16))
    hist = ec(nc.sbuf_tensor("hist_sb", [128, L, KC], dt.bfloat16))
    e0 = ec(nc.sbuf_tensor("e0_sb", [128, 1], dt.bfloat16))
    sigall = ec(nc.sbuf_tensor("sigall", [128, 6], dt.float32))
    tgc = ec(nc.sbuf_tensor("tgc", [128, 4], dt.float32))
    m12 = ec(nc.sbuf_tensor("m12", [128, 4], dt.float32))
    tcn = ec(nc.sbuf_tensor("tcn", [128, 2], dt.float32))
    hbuf = ec(nc.sbuf_tensor("hbuf", [128, 2, 2], dt.bfloat16))
    ysb = ec(nc.sbuf_tensor("ysb", [128, 256], dt.float32))
    pp0 = ec(nc.psum_tensor("pp0", [128, 8], dt.float32))
    pp1 = ec(nc.psum_tensor("pp1", [128, 8], dt.float32))
    yp = ec(nc.psum_tensor("yp", [128, 128], dt.float32))
    dsem = ec(nc.semaphore("dsem"))     # initial DMAs (HWDGE, x16)
    mmsem = ec(nc.semaphore("mmsem"))   # psum gates ready,   +1/step
    sa1 = ec(nc.semaphore("sa1"))       # sig+tanh_g ready,   +1/step
    sa2 = ec(nc.semaphore("sa2"))       # tanh_c ready,       +1/step
    sv1 = ec(nc.semaphore("sv1"))       # m12 ready,          +1/step
    sv2 = ec(nc.semaphore("sv2"))       # c updated,          +1/step
    sp1 = ec(nc.semaphore("sp1"))       # h written,          +1/step
    lsem0 = ec(nc.semaphore("lsem0"))   # queue-0 sends drained, +64/step
    lsem1 = ec(nc.semaphore("lsem1"))   # queue-1 sends drained, +64/step
    psem = ec(nc.semaphore("psem"))     # descs generated,    +8/step
    vfin = ec(nc.semaphore("vfin"))     # projection copies
    with nc.Block() as block:
        rsems = [stack.enter_context(nc.semaphore(f"rs{j}"))
                 for j in range(NC)]      # slot-j arrival, +2/step
        pp = [pp0, pp1]

        @block.tensor
        def _(te):
            te.wait_ge(dsem, 96)          # ALL initial DMAs (both wrec halves)
            for t in range(1, nsteps + 1):
                ps = pp[t % 2]
                if t >= 3:
                    te.wait_ge(sa2, t - 2)     # psum bank consumed (t-2)
                for m in range(8):             # bias chunk first (no dep)
                    te.matmul(ps[:, m:m + 1], wrec[:, KC, m, :], e0[:, 0:1],
                              start=(m == 0), stop=False)
                for j in range(NC):
                    if t >= 2:
                        te.wait_ge(rsems[j], 2 * (t - 1))
                    for i in range(2):
                        q = 2 * j + i
                        for m in range(8):
                            mm = te.matmul(
                                ps[:, m:m + 1], wrec[:, q, m, :],
                                hist[:, t - 1, q:q + 1],
                                start=False,
                                stop=(j == 7 and i == 1 and m == 7))
                mm.then_inc(mmsem, 1)
            # final projection: y[:, 128r:128r+128]
            for j in range(NC):
                te.wait_ge(rsems[j], 2 * nsteps)
            for tb in range(2):
                for q in range(KC):
                    mm = te.matmul(yp[:, :],
                                   hist[:, 128 * tb:128 * (tb + 1), q],
                                   wout[:, q, :],
                                   start=(q == 0), stop=(q == KC - 1))
                mm.then_inc(mmsem, 1)
                te.wait_ge(vfin, tb + 1)   # psum copied before reuse

        @block.scalar
        def _(sc):
            sc.dma_start(wrec[:, 9:17], wrec_d[:, 9:17]).then_inc(dsem, 16)
            for t in range(1, nsteps + 1):
                sc.wait_ge(mmsem, t)
                if t >= 2:
                    sc.wait_ge(sv1, t - 1)     # tgc[0:2] consumed by mul
                    sc.wait_ge(sp1, t - 1)     # sigall/tcn consumed by h-mul
                ps = pp[t % 2]
                # critical path first, as cheap [128,1]-column ACTs
                # (independent ops -> no same-engine RAW hazards)
                sc.activation(tgc[:, 0:1], ps[:, 6:7], Tanh)
                sc.activation(tgc[:, 1:2], ps[:, 7:8], Tanh)
                sc.activation(sigall[:, 0:1], ps[:, 0:1], Sig)
                sc.activation(sigall[:, 1:2], ps[:, 1:2], Sig)
                sc.activation(sigall[:, 2:3], ps[:, 2:3], Sig)
                sc.activation(sigall[:, 3:4], ps[:, 3:4], Sig) \
                  .then_inc(sa1, 1)
                # sig_o only feeds the late h-multiply: off the critical path
                sc.activation(sigall[:, 4:5], ps[:, 4:5], Sig)
                sc.activation(sigall[:, 5:6], ps[:, 5:6], Sig)
                sc.wait_ge(sv1, t)             # m12 ready
                sc.activation(tcn[:, 0:1], m12[:, 0:1], Tanh,
                              bias=m12[:, 2:3])
                sc.activation(tcn[:, 1:2], m12[:, 1:2], Tanh,
                              bias=m12[:, 3:4]).then_inc(sa2, 1)

        @block.vector
        def _(ve):
            ve.wait_ge(dsem, 96)               # all initial DMAs done
            for t in range(1, nsteps + 1):
                ve.wait_ge(sa1, t)
                if t >= 2:
                    ve.wait_ge(sv2, t - 1)     # same-engine RAW on tgc[2:4]
                    ve.wait_ge(sa2, t - 1)     # m12 consumed by tanh_c
                ve.tensor_mul(m12[:, :], sigall[:, 0:4], tgc[:, :]) \
                  .then_inc(sv1, 1)            # [i*tg | f*c]
                ve.wait_ge(sv1, t)             # same-engine RAW on m12
                ve.tensor_add(tgc[:, 2:4], m12[:, 0:2], m12[:, 2:4]) \
                  .then_inc(sv2, 1)            # c update (next step's f*c)
            # projection psum -> sbuf
            for tb in range(2):
                ve.wait_ge(mmsem, nsteps + tb + 1)
                ve.tensor_copy(ysb[:, 128 * tb:128 * (tb + 1)], yp[:, :]) \
                  .then_inc(vfin, 1)

        @block.gpsimd
        def _(gp):
            gp.load_library(library_config.remote_dma)
            for t in range(1, nsteps + 1):
                # desc-gen for this step's 8 sends (addresses only; runs
                # during the matvec). Slots 0-3 on queue 0, 4-7 on queue 1.
                for j in range(NC):
                    dests = [None] * NC
                    dests[j] = (0, j)
                    gp.remote_dma_broadcast(
                        hist[:, t, 2 * j:2 * j + 2],
                        hbuf[:, t % 2, :],
                        remote_sem=rsems[j],
                        local_sem=(lsem0 if j < 4 else lsem1),
                        rdests=dests,
                        queue_num=j // 4,
                    ).then_inc(psem, 1)
                # h = sig_o * tanh_c  (on Pool so the triggers that follow
                # need no cross-engine handshake)
                if t >= 3:
                    gp.wait_ge(lsem0, 64 * (t - 2))   # hbuf[t%2] drained
                    gp.wait_ge(lsem1, 64 * (t - 2))
                gp.wait_ge(sa2, t)
                gp.tensor_mul(hbuf[:, t % 2, :], sigall[:, 4:6], tcn[:, :]) \
                  .then_inc(sp1, 1)
                gp.wait_ge(sp1, t)      # all Q7 lanes of the mul committed
                gp.wait_ge(psem, 8 * t)
                gp.trigger_dma(count=4, queue_num=0)
                gp.trigger_dma(count=4, queue_num=1)

        @block.sync
        def _(sy):
            sy.dma_start(wrec[:, 0:9], wrec_d[:, 0:9]).then_inc(dsem, 16)
            sy.dma_start(e0[:], e0_d[:]).then_inc(dsem, 16)
            sy.dma_start(hist[:, 0, :], h1_d[:]).then_inc(dsem, 16)
            sy.dma_start(tgc[:, 2:4], c1_d[:]).then_inc(dsem, 16)
            sy.dma_start(wout[:], wout_d[:]).then_inc(dsem, 16)
            sy.wait_ge(vfin, 2)
            sy.dma_start(y_d[0:128, :], ysb[:, 0:128]).then_inc(dsem, 16)
            sy.dma_start(y_d[128:256, :], ysb[:, 128:256]).then_inc(dsem, 16)
            sy.wait_ge(dsem, 128)

    nc.compile()
    return nc


def kernel(x, W_ih, W_hh, b_ih, b_hh, W_out, b_out, seq_len, _trace=False):
    from concourse.bass_utils import run_bass_kernel_spmd

    assert int(seq_len) == L
    x = np.asarray(x, np.float32)[0]
    W_ih = np.asarray(W_ih, np.float32)
    W_sum = W_ih + np.asarray(W_hh, np.float32)
    b = np.asarray(b_ih, np.float32) + np.asarray(b_hh, np.float32)
    W_out = np.asarray(W_out, np.float32)
    b_out = np.asarray(b_out, np.float32)

    in_maps = _prep_core_inputs(x, W_ih, W_sum, b, W_out)
    nc = _build_program()
    res = run_bass_kernel_spmd(nc, in_maps, list(range(NC)), trace=_trace)
    y = np.concatenate(
        [np.asarray(res.results[r]["y"], np.float32) for r in range(NC)],
        axis=1)
    out = (y + b_out)[None]     # [1, L, DOUT]
    if _trace:
        return out, res
    return out


# revision 8
# speedup vs baseline: 1.7056x; 1.0074x over previous
"""Autoregressive LSTM decompressor on 8 Trainium2 NeuronCores, v2.

Math (from the reference): the output h of each step feeds back as the next
step's input, so for t>=1 the two matmuls collapse into one with
W = W_ih + W_hh:
    gates_0 = x @ W_ih.T + b            (h0 = c0 = 0; done on HOST, one matvec)
    gates_t = h_t @ W.T + b             (device, steps t=1..255)
    i,f,g,o = split(gates); c' = sig(f)*c + sig(i)*tanh(g); h' = sig(o)*tanh(c')
    y = stack(h_1..h_256) @ W_out.T + b_out

Device strategy (per step, tensor-parallel over the 4D gate dim):
  * Core r owns hidden units [256r, 256r+256) and the 1024 matching gate rows.
  * Gates are computed PARTITION-major: 136 matmuls with the weight tile
    [128h x 128gate] as the (free) stationary operand and the h chunk [128,1]
    as the moving operand -> psum [128, 8] = cols [i0 i1 f0 f1 o0 o1 g0 g1].
    Bias rides as a 17th contraction chunk against an e0 column.
  * Elementwise LSTM cell on [128,2] tiles across scalar/vector/pool engines.
  * h exchange: 8 single-destination remote_dma_broadcast sends per step
    (slot j -> physical peer my_tpb^j). Receiver-side slot s therefore holds
    the h chunk of logical core (s ^ r); each core's weight layout is
    permuted host-side to match. SBUF->SBUF, ~0.3us vs ~15us for a
    collective_compute AllGather. Per-slot arrival semaphores keep the
    consumer exact. hist slots are written once per step, never reused.
  * Final projection y = H @ W_out.T is sharded over output columns
    (core r computes douts [128r, 128r+128)).

Host-side prep is input formatting plus the single step-0 matvec (0.4% of
the FLOPs); all 255 recurrent steps and the projection run on device.
"""

import numpy as np
import ml_dtypes

D = 2048           # hidden width
DOUT = 1024        # output width
L = 256            # seq_len
NC = 8             # cores
KC = 16            # 128-wide contraction chunks per step
NSTEP = L - 1      # device steps (step 0 on host)

_BF16 = ml_dtypes.bfloat16

# psum column m -> gate type (rows in W are [i | f | g | o] blocks of 2048)
# col order   [i0 i1 f0 f1 o0 o1 g0 g1]  (sigmoid group contiguous 0:6)
_GT = np.array([0, 0, 1, 1, 3, 3, 2, 2])


# slot -> sender XOR map measured on HW: receiver r's hist slot-pair j holds
# the h chunk of logical core (r ^ SX[j]).  (Cross-die sends swap bit 1:
# the runtime's logical->physical NC map is the linear bit-matrix
# [bit2, bit1^bit2, bit0].)
SX = np.array([0, 1, 2, 3, 6, 7, 4, 5])


def _unit_map(r):
    """[128,16] global hidden unit of (partition p, hist chunk q) on core r."""
    P = np.arange(128)
    Q = np.arange(KC)
    gch = 2 * (SX[Q // 2] ^ r) + (Q % 2)        # global 128-chunk id
    return gch[None, :] * 128 + P[:, None]


def _row_map(r):
    """[8,128] W row of (psum col m, psum partition g) on core r."""
    M = np.arange(8)
    G = np.arange(128)
    return (2048 * _GT[M][:, None] + 256 * r + 128 * (M % 2)[:, None]
            + G[None, :])


def _prep_core_inputs(x, W_ih, W_sum, b, W_out):
    """Host: step-0 LSTM cell + per-core permuted weight uploads."""
    # step 0 on host in fp32 (h0 = c0 = 0)
    g0 = W_ih @ x + b
    i0, f0, gg0, o0 = np.split(g0, 4)
    c1 = _sig(i0) * np.tanh(gg0)
    h1 = _sig(o0) * np.tanh(c1)

    in_maps = []
    for r in range(NC):
        um = _unit_map(r)                       # [128,16]
        rm = _row_map(r)                        # [8,128]
        wrec = np.zeros((128, KC + 1, 8, 128), np.float32)
        # wrec[p,q,m,g] = W_sum[rm[m,g], um[p,q]]
        wsel = W_sum[rm.reshape(-1)][:, um.reshape(-1)]   # [1024, 2048]
        wrec[:, :KC] = (wsel.reshape(8, 128, 128, KC)
                        .transpose(2, 3, 0, 1))
        wrec[0, KC] = b[rm]                     # bias chunk (vs e0 column)
        wout = np.transpose(
            W_out[128 * r:128 * r + 128][:, um.reshape(-1)]
            .reshape(128, 128, KC), (1, 2, 0))  # [p, q, d]
        in_maps.append({
            "wrec": wrec.astype(_BF16),
            "wout": np.ascontiguousarray(wout).astype(_BF16),
            "h1": h1[um].astype(_BF16),         # [128,16]
            "c1": np.stack([c1[256 * r + np.arange(128)],
                            c1[256 * r + 128 + np.arange(128)]],
                           axis=1).astype(np.float32),    # [128,2]
            "e0": np.eye(128, 1, dtype=np.float32).astype(_BF16),
        })
    return in_maps


def _sig(v):
    return 1.0 / (1.0 + np.exp(-v))


def _build_program(nsteps=NSTEP):
    from concourse import bacc, mybir, library_config
    from contextlib import ExitStack

    dt = mybir.dt
    Sig = mybir.ActivationFunctionType.Sigmoid
    Tanh = mybir.ActivationFunctionType.Tanh

    nc = bacc.Bacc("TRN2", target_bir_lowering=False, debug=False,
                   num_devices=NC, num_swdge_queues=2)
    nc.detect_race_conditions = False

    wrec_d = nc.dram_tensor("wrec", [128, KC + 1, 8, 128], dt.bfloat16,
                            kind="ExternalInput")
    wout_d = nc.dram_tensor("wout", [128, KC, 128], dt.bfloat16,
                            kind="ExternalInput")
    h1_d = nc.dram_tensor("h1", [128, KC], dt.bfloat16, kind="ExternalInput")
    c1_d = nc.dram_tensor("c1", [128, 2], dt.float32, kind="ExternalInput")
    e0_d = nc.dram_tensor("e0", [128, 1], dt.bfloat16, kind="ExternalInput")
    y_d = nc.dram_tensor("y", [L, 128], dt.float32, kind="ExternalOutput")

    stack = ExitStack()
    ec = stack.enter_context
    wrec = ec(nc.sbuf_tensor("wrec_sb", [128, KC + 1, 8, 128], dt.bfloat16))
    wout = ec(nc.sbuf_tensor("wout_sb", [128, KC, 128], dt.bfloat16))
    hist = ec(nc.sbuf_tensor("hist_sb", [128, L, KC], dt.bfloat16))
    e0 = ec(nc.sbuf_tensor("e0_sb", [128, 1], dt.bfloat16))
    sigall = ec(nc.sbuf_tensor("sigall", [128, 6], dt.float32))
    tgc = ec(nc.sbuf_tensor("tgc", [128, 4], dt.float32))
    m12 = ec(nc.sbuf_tensor("m12", [128, 4], dt.float32))
    tcn = ec(nc.sbuf_tensor("tcn", [128, 2], dt.float32))
    hbuf = ec(nc.sbuf_tensor("hbuf", [128, 2, 2], dt.bfloat16))
    ysb = ec(nc.sbuf_tensor("ysb", [128, 256], dt.float32))
    pp0 = ec(nc.psum_tensor("pp0", [128, 8], dt.float32))
    pp1 = ec(nc.psum_tensor("pp1", [128, 8], dt.float32))
    yp = ec(nc.psum_tensor("yp", [128, 128], dt.float32))
    dsem = ec(nc.semaphore("dsem"))     # initial DMAs (HWDGE, x16)
    mmsem = ec(nc.semaphore("mmsem"))   # psum gates ready,   +1/step
    sa1 = ec(nc.semaphore("sa1"))       # sig+tanh_g ready,   +1/step
    sa2 = ec(nc.semaphore("sa2"))       # tanh_c ready,       +1/step
    sv1 = ec(nc.semaphore("sv1"))       # m12 ready,          +1/step
    sv2 = ec(nc.semaphore("sv2"))       # c updated,          +1/step
    sp1 = ec(nc.semaphore("sp1"))       # h written,          +1/step
    lsem0 = ec(nc.semaphore("lsem0"))   # queue-0 sends drained, +64/step
    lsem1 = ec(nc.semaphore("lsem1"))   # queue-1 sends drained, +64/step
    psem = ec(nc.semaphore("psem"))     # descs generated,    +8/step
    vfin = ec(nc.semaphore("vfin"))     # projection copies
    with nc.Block() as block:
        rsems = [stack.enter_context(nc.semaphore(f"rs{j}"))
                 for j in range(NC)]      # slot-j arrival, +2/step
        pp = [pp0, pp1]

        @block.tensor
        def _(te):
            te.wait_ge(dsem, 96)          # ALL initial DMAs (both wrec halves)
            for t in range(1, nsteps + 1):
                ps = pp[t % 2]
                if t >= 3:
                    te.wait_ge(sa2, t - 2)     # psum bank consumed (t-2)
                for m in range(8):             # bias chunk first (no dep)
                    te.matmul(ps[:, m:m + 1], wrec[:, KC, m, :], e0[:, 0:1],
                              start=(m == 0), stop=False)
                for j in range(NC):
                    if t >= 2:
                        te.wait_ge(rsems[j], 2 * (t - 1))
                    for i in range(2):
                        q = 2 * j + i
                        for m in range(8):
                            mm = te.matmul(
                                ps[:, m:m + 1], wrec[:, q, m, :],
                                hist[:, t - 1, q:q + 1],
                                start=False,
                                stop=(j == 7 and i == 1 and m == 7))
                mm.then_inc(mmsem, 1)
            # final projection: y[:, 128r:128r+128]
            for j in range(NC):
                te.wait_ge(rsems[j], 2 * nsteps)
            for tb in range(2):
                for q in range(KC):
                    mm = te.matmul(yp[:, :],
                                   hist[:, 128 * tb:128 * (tb + 1), q],
                                   wout[:, q, :],
                                   start=(q == 0), stop=(q == KC - 1))
                mm.then_inc(mmsem, 1)
                te.wait_ge(vfin, tb + 1)   # psum copied before reuse

        @block.scalar
        def _(sc):
            sc.dma_start(wrec[:, 9:17], wrec_d[:, 9:17]).then_inc(dsem, 16)
            sc.dma_start(wout[:], wout_d[:]).then_inc(dsem, 16)
            for t in range(1, nsteps + 1):
                sc.wait_ge(mmsem, t)
                if t >= 2:
                    sc.wait_ge(sv1, t - 1)     # tgc[0:2] consumed by mul
                    sc.wait_ge(sp1, t - 1)     # sigall/tcn consumed by h-mul
                ps = pp[t % 2]
                # critical path first, as cheap [128,1]-column ACTs
                # (independent ops -> no same-engine RAW hazards)
                sc.activation(tgc[:, 0:1], ps[:, 6:7], Tanh)
                sc.activation(tgc[:, 1:2], ps[:, 7:8], Tanh)
                sc.activation(sigall[:, 0:1], ps[:, 0:1], Sig)
                sc.activation(sigall[:, 1:2], ps[:, 1:2], Sig)
                sc.activation(sigall[:, 2:3], ps[:, 2:3], Sig)
                sc.activation(sigall[:, 3:4], ps[:, 3:4], Sig) \
                  .then_inc(sa1, 1)
                # sig_o only feeds the late h-multiply: off the critical path
                sc.activation(sigall[:, 4:5], ps[:, 4:5], Sig)
                sc.activation(sigall[:, 5:6], ps[:, 5:6], Sig)
                sc.wait_ge(sv1, t)             # m12 ready
                sc.activation(tcn[:, 0:1], m12[:, 0:1], Tanh,
                              bias=m12[:, 2:3])
                sc.activation(tcn[:, 1:2], m12[:, 1:2], Tanh,
                              bias=m12[:, 3:4]).then_inc(sa2, 1)

        @block.vector
        def _(ve):
            ve.wait_ge(dsem, 96)               # all initial DMAs done
            for t in range(1, nsteps + 1):
                ve.wait_ge(sa1, t)
                if t >= 2:
                    ve.wait_ge(sv2, t - 1)     # same-engine RAW on tgc[2:4]
                    ve.wait_ge(sa2, t - 1)     # m12 consumed by tanh_c
                ve.tensor_mul(m12[:, :], sigall[:, 0:4], tgc[:, :]) \
                  .then_inc(sv1, 1)            # [i*tg | f*c]
                ve.wait_ge(sv1, t)             # same-engine RAW on m12
                ve.tensor_add(tgc[:, 2:4], m12[:, 0:2], m12[:, 2:4]) \
                  .then_inc(sv2, 1)            # c update (next step's f*c)
            # projection psum -> sbuf
            for tb in range(2):
                ve.wait_ge(mmsem, nsteps + tb + 1)
                ve.tensor_copy(ysb[:, 128 * tb:128 * (tb + 1)], yp[:, :]) \
                  .then_inc(vfin, 1)

        @block.gpsimd
        def _(gp):
            gp.load_library(library_config.remote_dma)
            for t in range(1, nsteps + 1):
                # desc-gen for this step's 8 sends (addresses only; runs
                # during the matvec). Slots 0-3 on queue 0, 4-7 on queue 1.
                for j in range(NC):
                    dests = [None] * NC
                    dests[j] = (0, j)
                    gp.remote_dma_broadcast(
                        hist[:, t, 2 * j:2 * j + 2],
                        hbuf[:, t % 2, :],
                        remote_sem=rsems[j],
                        local_sem=(lsem0 if j < 4 else lsem1),
                        rdests=dests,
                        queue_num=j // 4,
                    ).then_inc(psem, 1)
                # h = sig_o * tanh_c  (on Pool so the triggers that follow
                # need no cross-engine handshake)
                if t >= 3:
                    gp.wait_ge(lsem0, 64 * (t - 2))   # hbuf[t%2] drained
                    gp.wait_ge(lsem1, 64 * (t - 2))
                gp.wait_ge(sa2, t)
                gp.tensor_mul(hbuf[:, t % 2, :], sigall[:, 4:6], tcn[:, :]) \
                  .then_inc(sp1, 1)
                gp.wait_ge(sp1, t)      # all Q7 lanes of the mul committed
                gp.wait_ge(psem, 8 * t)
                gp.trigger_dma(count=4, queue_num=0)
                gp.trigger_dma(count=4, queue_num=1)

        @block.sync
        def _(sy):
            sy.dma_start(wrec[:, 0:9], wrec_d[:, 0:9]).then_inc(dsem, 16)
            sy.dma_start(e0[:], e0_d[:]).then_inc(dsem, 16)
            sy.dma_start(hist[:, 0, :], h1_d[:]).then_inc(dsem, 16)
            sy.dma_start(tgc[:, 2:4], c1_d[:]).then_inc(dsem, 16)
            sy.wait_ge(vfin, 2)
            sy.dma_start(y_d[0:128, :], ysb[:, 0:128]).then_inc(dsem, 16)
            sy.dma_start(y_d[128:256, :], ysb[:, 128:256]).then_inc(dsem, 16)
            sy.wait_ge(dsem, 128)

    nc.compile()
    return nc


def kernel(x, W_ih, W_hh, b_ih, b_hh, W_out, b_out, seq_len, _trace=False):
    from concourse.bass_utils import run_bass_kernel_spmd

    assert int(seq_len) == L
    x = np.asarray(x, np.float32)[0]
    W_ih = np.asarray(W_ih, np.float32)
    W_sum = W_ih + np.asarray(W_hh, np.float32)
    b = np.asarray(b_ih, np.float32) + np.asarray(b_hh, np.float32)
    W_out = np.asarray(W_out, np.float32)
    b_out = np.asarray(b_out, np.float32)

    in_maps = _prep_core_inputs(x, W_ih, W_sum, b, W_out)
    nc = _build_program()
    res = run_bass_kernel_spmd(nc, in_maps, list(range(NC)), trace=_trace)
    y = np.concatenate(
        [np.asarray(res.results[r]["y"], np.float32) for r in range(NC)],
        axis=1)
    out = (y + b_out)[None]     # [1, L, DOUT]
    if _trace:
        return out, res
    return out
